# revision 1
# baseline (speedup 1.0000x reference)
"""Hamiltonian block-generation layer on 8 Trainium2 NeuronCores.

Strategy: shard the pair dimension P=130816 across 8 cores (16352 pairs each,
padded to 16384). The host pre-gathers transposed MLP inputs
xT = concat(n_i, n_j, e_ij)^T  [384, 16384] per core, so the device kernel is
identical on every core (pure SPMD):

  stage 1:  hoT[h, p] = silu(Wo1^T @ xT + bo1)        (2x matmul-accum chains)
  stage 2:  out_u = hoT^T @ Wo2  + (overlap + bo2)     (i,j block, row-major)
            out_t = hoT^T @ Wo2p + (overlap^T + bo2p)  (j,i block = transposed
                                                        block, via host-permuted
                                                        second-layer weights)
  diag:     batched separately (64 atoms/core) with W1/W2.

Outputs stay in pair-block layout [16384, 196] (perfectly coalesced DMA); the
host scatters blocks into the dense H [7168, 7168].
"""

import numpy as np
import ml_dtypes

BF16 = ml_dtypes.bfloat16

N_ATOMS = 512
B = 14
BB = B * B          # 196
F = 128
FE = 128
HID = 256
P = N_ATOMS * (N_ATOMS - 1) // 2   # 130816
NCORES = 8
PPC = P // NCORES                  # 16352 pairs per core
NB = 512                           # pairs per batch
NBATCH = (PPC + NB - 1) // NB      # 32
PPCP = NBATCH * NB                 # 16384 padded
DPC = N_ATOMS // NCORES            # 64 diag atoms per core

_CACHE = {}


def _build_nc():
    import concourse.mybir as mybir
    import concourse.tile as tile
    from concourse import bacc

    f32 = mybir.dt.float32
    nc = bacc.Bacc("TRN2", target_bir_lowering=False)

    bf16 = mybir.dt.bfloat16
    xT = nc.dram_tensor("xT", [3 * F, PPCP], bf16, kind="ExternalInput")
    ovu = nc.dram_tensor("ovu", [PPCP, BB], f32, kind="ExternalInput")
    xdT = nc.dram_tensor("xdT", [HID, DPC], bf16, kind="ExternalInput")
    apd = nc.dram_tensor("apd", [DPC, BB], f32, kind="ExternalInput")
    Wo1 = nc.dram_tensor("Wo1", [3 * F, HID], bf16, kind="ExternalInput")
    W1 = nc.dram_tensor("W1", [HID, HID], bf16, kind="ExternalInput")
    Wo2c = nc.dram_tensor("Wo2c", [HID, 2 * BB], bf16, kind="ExternalInput")
    W2 = nc.dram_tensor("W2", [HID, BB], bf16, kind="ExternalInput")
    b1 = nc.dram_tensor("b1", [2, 128], f32, kind="ExternalInput")
    bo1 = nc.dram_tensor("bo1", [2, 128], f32, kind="ExternalInput")

    out_u = nc.dram_tensor("out_u", [PPCP, BB], f32, kind="ExternalOutput")
    out_t = nc.dram_tensor("out_t", [PPCP, BB], f32, kind="ExternalOutput")
    out_d = nc.dram_tensor("out_d", [DPC, BB], f32, kind="ExternalOutput")

    import concourse.bass as bass  # noqa: F401

    silu = mybir.ActivationFunctionType.Silu

    with tile.TileContext(nc) as tc:
        with tc.tile_pool(name="consts", bufs=1) as consts, \
             tc.tile_pool(name="xin", bufs=3) as xin, \
             tc.tile_pool(name="hpool", bufs=2) as hpool, \
             tc.tile_pool(name="ovp", bufs=3) as ovp, \
             tc.tile_pool(name="outp", bufs=3) as outp, \
             tc.tile_pool(name="psH", bufs=2, space="PSUM") as psH, \
             tc.tile_pool(name="psO", bufs=4, space="PSUM") as psO:

            # ---- weights into SBUF, K-chunked: [128, nchunk, out_dim]
            wo1 = consts.tile([128, 3, HID], bf16, tag="wo1")
            nc.sync.dma_start(out=wo1, in_=Wo1.rearrange("(c p) h -> p c h", p=128))
            w1 = consts.tile([128, 2, HID], bf16, tag="w1")
            nc.sync.dma_start(out=w1, in_=W1.rearrange("(c p) h -> p c h", p=128))
            wo2c = consts.tile([128, 2, 2 * BB], bf16, tag="wo2c")
            nc.sync.dma_start(out=wo2c, in_=Wo2c.rearrange("(c p) e -> p c e", p=128))
            w2 = consts.tile([128, 2, BB], bf16, tag="w2")
            nc.sync.dma_start(out=w2, in_=W2.rearrange("(c p) e -> p c e", p=128))
            b1t = consts.tile([128, 2], f32, tag="b1t")
            nc.sync.dma_start(out=b1t, in_=b1.rearrange("c p -> p c"))
            bo1t = consts.tile([128, 2], f32, tag="bo1t")
            nc.sync.dma_start(out=bo1t, in_=bo1.rearrange("c p -> p c"))

            # ---- main pair loop
            for bi in range(NBATCH):
                base = bi * NB
                rhs = []
                for k in range(3):
                    r = xin.tile([128, NB], bf16, tag=f"rhs{k}")
                    nc.sync.dma_start(
                        out=r, in_=xT[k * 128:(k + 1) * 128, base:base + NB])
                    rhs.append(r)
                ho = []
                for h in range(2):
                    ph = psH.tile([128, NB], f32, tag=f"psh{h}")
                    for k in range(3):
                        nc.tensor.matmul(
                            ph, wo1[:, k, h * 128:(h + 1) * 128], rhs[k],
                            start=(k == 0), stop=(k == 2))
                    hs = hpool.tile([128, NB], bf16, tag=f"ho{h}")
                    nc.scalar.activation(hs, ph, silu, bias=bo1t[:, h:h + 1])
                    ho.append(hs)
                for pt in range(NB // 128):
                    row = base + pt * 128
                    ov = ovp.tile([128, BB], f32, tag="ov")
                    nc.sync.dma_start(out=ov, in_=ovu[row:row + 128, :])
                    ps = psO.tile([128, 2 * BB], f32, tag="pso")
                    for h in range(2):
                        nc.tensor.matmul(
                            ps, ho[h][:, pt * 128:(pt + 1) * 128],
                            wo2c[:, h, :], start=(h == 0), stop=(h == 1))
                    for (off, o_dram, swapped) in (
                            (0, out_u, False), (BB, out_t, True)):
                        if swapped:
                            ov_in = ov[:, :].rearrange("p (v u) -> p u v", v=B)
                        else:
                            ov_in = ov[:, :].rearrange("p (u v) -> p u v", u=B)
                        ot = outp.tile([128, BB], f32, tag="ot")
                        nc.vector.tensor_add(
                            ot[:, :].rearrange("p (u v) -> p u v", u=B),
                            ps[:, off:off + BB].rearrange("p (u v) -> p u v", u=B),
                            ov_in)
                        nc.sync.dma_start(out=o_dram[row:row + 128, :], in_=ot)

            # ---- diagonal blocks (64 atoms)
            rd = xin.tile([128, 2, DPC], bf16, tag="rhsd")
            nc.sync.dma_start(out=rd, in_=xdT.rearrange("(c p) a -> p c a", p=128))
            hod = []
            for h in range(2):
                ph = psH.tile([128, DPC], f32, tag=f"psh{h}")
                for k in range(2):
                    nc.tensor.matmul(
                        ph, w1[:, k, h * 128:(h + 1) * 128], rd[:, k, :],
                        start=(k == 0), stop=(k == 1))
                hs = hpool.tile([128, DPC], bf16, tag=f"hod{h}")
                nc.scalar.activation(hs, ph, silu, bias=b1t[:, h:h + 1])
                hod.append(hs)
            psd = psO.tile([DPC, BB], f32, tag="pso")
            for h in range(2):
                nc.tensor.matmul(psd, hod[h], w2[:, h, :],
                                 start=(h == 0), stop=(h == 1))
            apt = ovp.tile([DPC, BB], f32, tag="ov")
            nc.sync.dma_start(out=apt, in_=apd[:, :])
            otd = outp.tile([DPC, BB], f32, tag="ot")
            nc.vector.tensor_add(otd, psd, apt)
            nc.sync.dma_start(out=out_d[:, :], in_=otd)

    nc.finalize()
    return nc


def kernel(**inputs) -> np.ndarray:
    nodes_feature = np.ascontiguousarray(np.asarray(inputs["nodes_feature"], np.float32))
    edges_feature = np.asarray(inputs["edges_feature"], np.float32)
    atom_blocks = np.asarray(inputs["atom_blocks"], np.float32)
    overlap_pair = np.asarray(inputs["overlap_pair"], np.float32)
    W1 = np.ascontiguousarray(np.asarray(inputs["W1"], np.float32))
    b1 = np.asarray(inputs["b1"], np.float32)
    W2 = np.ascontiguousarray(np.asarray(inputs["W2"], np.float32))
    b2 = np.asarray(inputs["b2"], np.float32)
    Wo1 = np.ascontiguousarray(np.asarray(inputs["Wo1"], np.float32))
    bo1 = np.asarray(inputs["bo1"], np.float32)
    Wo2 = np.ascontiguousarray(np.asarray(inputs["Wo2"], np.float32))
    bo2 = np.asarray(inputs["bo2"], np.float32)
    pair_i = np.asarray(inputs["pair_i"]).astype(np.int64)
    pair_j = np.asarray(inputs["pair_j"]).astype(np.int64)

    # ---- host prep
    nodesT = np.ascontiguousarray(nodes_feature.T)                 # [128, 512]
    e = np.arange(BB)
    perm = (e % B) * B + e // B                                    # transpose perm
    Wo2c = np.ascontiguousarray(
        np.concatenate([Wo2, Wo2[:, perm]], axis=1)).astype(BF16)
    bo2p = bo2[perm]
    ar = np.arange(N_ATOMS)
    eaa = edges_feature[ar, ar]                                    # [512, 128]

    in_maps = []
    for m in range(NCORES):
        sel = slice(m * PPC, (m + 1) * PPC)
        pi, pj = pair_i[sel], pair_j[sel]
        xT = np.zeros((3 * F, PPCP), BF16)
        xT[0:128, :PPC] = nodesT[:, pi].astype(BF16)
        xT[128:256, :PPC] = nodesT[:, pj].astype(BF16)
        xT[256:384, :PPC] = edges_feature[pi, pj].T.astype(BF16)
        ovu = np.zeros((PPCP, BB), np.float32)
        ovu[:PPC] = overlap_pair[sel].reshape(-1, BB) + bo2
        d = slice(m * DPC, (m + 1) * DPC)
        xdT = np.empty((HID, DPC), np.float32)
        xdT[0:128] = nodesT[:, d]
        xdT[128:256] = eaa[d].T
        apd = atom_blocks[d].reshape(-1, BB) + b2
        in_maps.append({
            "xT": xT, "ovu": ovu,
            "xdT": np.ascontiguousarray(xdT).astype(BF16),
            "apd": np.ascontiguousarray(apd),
            "Wo1": Wo1.astype(BF16), "W1": W1.astype(BF16),
            "Wo2c": Wo2c, "W2": W2.astype(BF16),
            "b1": np.ascontiguousarray(b1.reshape(2, 128)),
            "bo1": np.ascontiguousarray(bo1.reshape(2, 128)),
        })

    if "nc" not in _CACHE:
        _CACHE["nc"] = _build_nc()
    nc = _CACHE["nc"]

    import os
    import time
    from concourse.bass_utils import run_bass_kernel_spmd
    trace = bool(int(os.environ.get("KERNEL_TRACE", "0")))
    t0 = time.time()
    if trace:
        try:
            res = run_bass_kernel_spmd(nc, in_maps, core_ids=list(range(NCORES)),
                                       trace=True)
        except Exception:
            res = run_bass_kernel_spmd(nc, in_maps, core_ids=list(range(NCORES)))
    else:
        res = run_bass_kernel_spmd(nc, in_maps, core_ids=list(range(NCORES)))
    _CACHE["run_wall_s"] = time.time() - t0
    _CACHE["last_result"] = res

    # ---- host scatter into dense H
    H4 = np.zeros((N_ATOMS, B, N_ATOMS, B), np.float32)
    all_u = np.concatenate([res.results[m]["out_u"][:PPC] for m in range(NCORES)])
    all_t = np.concatenate([res.results[m]["out_t"][:PPC] for m in range(NCORES)])
    all_d = np.concatenate([res.results[m]["out_d"] for m in range(NCORES)])
    H4[pair_i, :, pair_j, :] = all_u.reshape(-1, B, B)
    H4[pair_j, :, pair_i, :] = all_t.reshape(-1, B, B)
    ar = np.arange(N_ATOMS)
    H4[ar, :, ar, :] = all_d.reshape(-1, B, B)
    return H4.reshape(N_ATOMS * B, N_ATOMS * B)



# revision 2
# speedup vs baseline: 6.2731x; 6.2731x over previous
"""Hamiltonian block-generation layer on 8 Trainium2 NeuronCores.

The axon tunnel (~45 MB/s up, ~35 MB/s down) dominates wall time, so the
design minimizes transferred bytes:

  - pair dim P=130816 sharded 8 ways (16352/core, padded to 16384)
  - node features are gathered ON DEVICE (gpsimd indirect_copy) from a tiny
    replicated nodesT [128, 512] bf16 using uint16 pair indices, instead of
    shipping pre-gathered [256, 16384] slabs per core
  - edge features e_ij are host-gathered per core and shipped as fp8e4
    transport ([128, 16384] = 2.1 MB/core); upcast to bf16 on device before
    the matmul (no fp8 matmul)
  - overlap/bias adds and the block scatter happen on the host, so the
    device returns only the raw MLP output, fp8e4 at scale 32
    ([16448, 196] = 3.2 MB/core, single output tensor)

Device math per core (32 batches of 512 pairs):
  x = [gather(nodesT, i); gather(nodesT, j); edge]   (bf16, K=384)
  h = silu(Wo1^T @ x + bo1)                          (bf16, HID=256)
  mo = (h^T @ Wo2) * 32 -> fp8                       (196 per pair)
plus 64 diagonal atoms/core through W1/W2 the same way.
"""

import numpy as np
import ml_dtypes

BF16 = ml_dtypes.bfloat16
F8 = ml_dtypes.float8_e4m3

N_ATOMS = 512
B = 14
BB = B * B          # 196
F = 128
FE = 128
HID = 256
P = N_ATOMS * (N_ATOMS - 1) // 2   # 130816
NCORES = 8
PPC = P // NCORES                  # 16352 pairs per core
NB = 512                           # pairs per batch
NBATCH = (PPC + NB - 1) // NB      # 32
PPCP = NBATCH * NB                 # 16384 padded
DPC = N_ATOMS // NCORES            # 64 diag atoms per core
OSCALE = 32.0                      # fp8 transport scale for MLP outputs

_CACHE = {}


def _build_nc():
    import concourse.mybir as mybir
    import concourse.tile as tile
    from concourse import bacc

    f32 = mybir.dt.float32
    bf16 = mybir.dt.bfloat16
    f8 = mybir.dt.float8e4
    u16 = mybir.dt.uint16
    nc = bacc.Bacc("TRN2", target_bir_lowering=False)

    nodesTf = nc.dram_tensor("nodesTf", [F, N_ATOMS], bf16, kind="ExternalInput")
    idxi = nc.dram_tensor("idxi", [128, PPCP // 16], u16, kind="ExternalInput")
    idxj = nc.dram_tensor("idxj", [128, PPCP // 16], u16, kind="ExternalInput")
    edge = nc.dram_tensor("edge", [FE, PPCP], f8, kind="ExternalInput")
    xdT = nc.dram_tensor("xdT", [HID, DPC], bf16, kind="ExternalInput")
    Wo1 = nc.dram_tensor("Wo1", [3 * F, HID], bf16, kind="ExternalInput")
    W1 = nc.dram_tensor("W1", [HID, HID], bf16, kind="ExternalInput")
    Wo2 = nc.dram_tensor("Wo2", [HID, BB], bf16, kind="ExternalInput")
    W2 = nc.dram_tensor("W2", [HID, BB], bf16, kind="ExternalInput")
    b1 = nc.dram_tensor("b1", [2, 128], f32, kind="ExternalInput")
    bo1 = nc.dram_tensor("bo1", [2, 128], f32, kind="ExternalInput")

    mo = nc.dram_tensor("mo", [PPCP + DPC, BB], f8, kind="ExternalOutput")

    import os
    if os.environ.get("KERNEL_ACT") == "sigmoid":
        # the CPU simulator does not implement Silu; test_sim.py swaps in
        # Sigmoid (and compares against a sigmoid-based numpy model) to
        # validate everything else
        silu = mybir.ActivationFunctionType.Sigmoid
    else:
        silu = mybir.ActivationFunctionType.Silu
    copyf = mybir.ActivationFunctionType.Copy

    with tile.TileContext(nc) as tc:
        with tc.tile_pool(name="consts", bufs=1) as consts, \
             tc.tile_pool(name="gat", bufs=3) as gat, \
             tc.tile_pool(name="xin", bufs=3) as xin, \
             tc.tile_pool(name="hpool", bufs=2) as hpool, \
             tc.tile_pool(name="outp", bufs=4) as outp, \
             tc.tile_pool(name="psH", bufs=2, space="PSUM") as psH, \
             tc.tile_pool(name="psO", bufs=4, space="PSUM") as psO:

            # ---- persistent SBUF state
            nt = consts.tile([128, N_ATOMS], bf16, tag="nt")
            nc.sync.dma_start(out=nt, in_=nodesTf[:, :])
            ii = consts.tile([128, PPCP // 16], u16, tag="ii")
            nc.sync.dma_start(out=ii, in_=idxi[:, :])
            jj = consts.tile([128, PPCP // 16], u16, tag="jj")
            nc.sync.dma_start(out=jj, in_=idxj[:, :])
            ed = consts.tile([128, PPCP], f8, tag="ed")
            nc.sync.dma_start(out=ed, in_=edge[:, :])
            wo1 = consts.tile([128, 3, HID], bf16, tag="wo1")
            nc.sync.dma_start(out=wo1, in_=Wo1.rearrange("(c p) h -> p c h", p=128))
            w1 = consts.tile([128, 2, HID], bf16, tag="w1")
            nc.sync.dma_start(out=w1, in_=W1.rearrange("(c p) h -> p c h", p=128))
            wo2 = consts.tile([128, 2, BB], bf16, tag="wo2")
            nc.sync.dma_start(out=wo2, in_=Wo2.rearrange("(c p) e -> p c e", p=128))
            w2 = consts.tile([128, 2, BB], bf16, tag="w2")
            nc.sync.dma_start(out=w2, in_=W2.rearrange("(c p) e -> p c e", p=128))
            b1t = consts.tile([128, 2], f32, tag="b1t")
            nc.sync.dma_start(out=b1t, in_=b1.rearrange("c p -> p c"))
            bo1t = consts.tile([128, 2], f32, tag="bo1t")
            nc.sync.dma_start(out=bo1t, in_=bo1.rearrange("c p -> p c"))

            # ---- main pair loop
            for bi in range(NBATCH):
                base = bi * NB
                isl = slice(bi * (NB // 16), (bi + 1) * (NB // 16))
                xg = []
                for (name, idxt) in (("xi", ii), ("xj", jj)):
                    xb = xin.tile([128, NB], bf16, tag=f"b_{name}")
                    nc.gpsimd.indirect_copy(
                        xb, nt, idxt[:, isl],
                        i_know_ap_gather_is_preferred=True)
                    xg.append(xb)
                eb = xin.tile([128, NB], bf16, tag="b_e")
                nc.vector.tensor_copy(eb, ed[:, base:base + NB])
                rhs = [xg[0], xg[1], eb]
                ho = []
                for h in range(2):
                    ph = psH.tile([128, NB], f32, tag=f"psh{h}")
                    for k in range(3):
                        nc.tensor.matmul(
                            ph, wo1[:, k, h * 128:(h + 1) * 128], rhs[k],
                            start=(k == 0), stop=(k == 2))
                    hs = hpool.tile([128, NB], bf16, tag=f"ho{h}")
                    nc.scalar.activation(hs, ph, silu, bias=bo1t[:, h:h + 1])
                    ho.append(hs)
                for pt in range(NB // 128):
                    row = base + pt * 128
                    ps = psO.tile([128, BB], f32, tag="pso")
                    for h in range(2):
                        nc.tensor.matmul(
                            ps, ho[h][:, pt * 128:(pt + 1) * 128],
                            wo2[:, h, :], start=(h == 0), stop=(h == 1))
                    ot = outp.tile([128, BB], f8, tag="ot")
                    nc.scalar.activation(ot, ps, copyf, scale=OSCALE)
                    nc.sync.dma_start(out=mo[row:row + 128, :], in_=ot)

            # ---- diagonal blocks (64 atoms/core)
            rd = xin.tile([128, 2, DPC], bf16, tag="rhsd")
            nc.sync.dma_start(out=rd, in_=xdT.rearrange("(c p) a -> p c a", p=128))
            hod = []
            for h in range(2):
                ph = psH.tile([128, DPC], f32, tag=f"psh{h}")
                for k in range(2):
                    nc.tensor.matmul(
                        ph, w1[:, k, h * 128:(h + 1) * 128], rd[:, k, :],
                        start=(k == 0), stop=(k == 1))
                hs = hpool.tile([128, DPC], bf16, tag=f"hod{h}")
                nc.scalar.activation(hs, ph, silu, bias=b1t[:, h:h + 1])
                hod.append(hs)
            psd = psO.tile([DPC, BB], f32, tag="pso")
            for h in range(2):
                nc.tensor.matmul(psd, hod[h], w2[:, h, :],
                                 start=(h == 0), stop=(h == 1))
            otd = outp.tile([DPC, BB], f8, tag="otd")
            nc.scalar.activation(otd, psd, copyf, scale=OSCALE)
            nc.sync.dma_start(out=mo[PPCP:PPCP + DPC, :], in_=otd)

    nc.finalize()
    return nc


def _wrap_idx(idx_padded):
    # indirect_copy index layout: output position k of a 16-partition group
    # reads the index stored at partition k%16, free column k//16 (wrapped),
    # and the layout is replicated across the 8 groups.
    blk = idx_padded.reshape(NBATCH, NB // 16, 16)          # [bi, s, p]
    w = blk.transpose(2, 0, 1).reshape(16, PPCP // 16)      # [p, bi*32+s]
    return np.ascontiguousarray(np.tile(w, (8, 1)))         # [128, PPCP//16]


def kernel(**inputs) -> np.ndarray:
    nodes_feature = np.ascontiguousarray(np.asarray(inputs["nodes_feature"], np.float32))
    edges_feature = np.asarray(inputs["edges_feature"], np.float32)
    atom_blocks = np.asarray(inputs["atom_blocks"], np.float32)
    overlap_pair = np.asarray(inputs["overlap_pair"], np.float32)
    W1 = np.ascontiguousarray(np.asarray(inputs["W1"], np.float32))
    b1 = np.asarray(inputs["b1"], np.float32)
    W2 = np.ascontiguousarray(np.asarray(inputs["W2"], np.float32))
    b2 = np.asarray(inputs["b2"], np.float32)
    Wo1 = np.ascontiguousarray(np.asarray(inputs["Wo1"], np.float32))
    bo1 = np.asarray(inputs["bo1"], np.float32)
    Wo2 = np.ascontiguousarray(np.asarray(inputs["Wo2"], np.float32))
    bo2 = np.asarray(inputs["bo2"], np.float32)
    pair_i = np.asarray(inputs["pair_i"]).astype(np.int64)
    pair_j = np.asarray(inputs["pair_j"]).astype(np.int64)

    # ---- host prep
    nodesT = np.ascontiguousarray(nodes_feature.T)                 # [128, 512]
    ar = np.arange(N_ATOMS)
    eaa = edges_feature[ar, ar]                                    # [512, 128]
    Wo1b = Wo1.astype(BF16)
    W1b = W1.astype(BF16)
    Wo2b = Wo2.astype(BF16)
    W2b = W2.astype(BF16)
    b1r = np.ascontiguousarray(b1.reshape(2, 128))
    bo1r = np.ascontiguousarray(bo1.reshape(2, 128))

    in_maps = []
    for m in range(NCORES):
        sel = slice(m * PPC, (m + 1) * PPC)
        pi, pj = pair_i[sel], pair_j[sel]
        pip = np.zeros(PPCP, np.uint16)
        pjp = np.zeros(PPCP, np.uint16)
        pip[:PPC] = pi
        pjp[:PPC] = pj
        eg = np.zeros((FE, PPCP), F8)
        eg[:, :PPC] = edges_feature[pi, pj].T.astype(F8)
        d = slice(m * DPC, (m + 1) * DPC)
        xdT = np.empty((HID, DPC), np.float32)
        xdT[0:128] = nodesT[:, d]
        xdT[128:256] = eaa[d].T
        in_maps.append({
            "nodesTf": nodesT.astype(BF16),
            "idxi": _wrap_idx(pip), "idxj": _wrap_idx(pjp),
            "edge": eg,
            "xdT": np.ascontiguousarray(xdT).astype(BF16),
            "Wo1": Wo1b, "W1": W1b, "Wo2": Wo2b, "W2": W2b,
            "b1": b1r, "bo1": bo1r,
        })

    if "nc" not in _CACHE:
        _CACHE["nc"] = _build_nc()
    nc = _CACHE["nc"]

    import os
    import time
    from concourse.bass_utils import run_bass_kernel_spmd
    trace = bool(int(os.environ.get("KERNEL_TRACE", "0")))
    t0 = time.time()
    if trace:
        try:
            res = run_bass_kernel_spmd(nc, in_maps, core_ids=list(range(NCORES)),
                                       trace=True)
        except Exception:
            res = run_bass_kernel_spmd(nc, in_maps, core_ids=list(range(NCORES)))
    else:
        res = run_bass_kernel_spmd(nc, in_maps, core_ids=list(range(NCORES)))
    _CACHE["run_wall_s"] = time.time() - t0
    _CACHE["last_result"] = res

    # ---- host epilogue: add overlap/bias, scatter blocks into dense H
    inv = np.float32(1.0 / OSCALE)
    all_mo = np.concatenate(
        [res.results[m]["mo"][:PPC] for m in range(NCORES)]).astype(np.float32)
    off = overlap_pair.reshape(P, BB) + bo2 + all_mo * inv         # [P, 196]
    off = off.reshape(P, B, B)
    diag = np.concatenate(
        [res.results[m]["mo"][PPCP:PPCP + DPC] for m in range(NCORES)]
    ).astype(np.float32)
    diag = atom_blocks + (b2 + diag * inv).reshape(N_ATOMS, B, B)

    H4 = np.zeros((N_ATOMS, B, N_ATOMS, B), np.float32)
    H4[pair_i, :, pair_j, :] = off
    H4[pair_j, :, pair_i, :] = off.transpose(0, 2, 1)
    H4[ar, :, ar, :] = diag
    return H4.reshape(N_ATOMS * B, N_ATOMS * B)
